# revision 26
# baseline (speedup 1.0000x reference)
"""TRN2 Bass kernel for nn_Attention_78348793414287 (linear attention).

Reference computation (N=4, T=4096, H=16, DM=DA=1024, dh=64; masks all-ones):
  qh = split_heads(q @ Wq); kh = split_heads(k @ Wk); vh = split_heads(v @ Wv)
  k_sm = softmax(kh, axis=t);  kv = einsum('nhtd,nhte->nhde', k_sm, vh)
  q_sm = softmax(qh, axis=d);  out = einsum('nhtd,nhte->nhte', q_sm, kv)

Sharding: 8 cores = 4 batches x 2 head-groups (8 heads / 512 cols per core),
no collectives; host shards inputs and gathers outputs.

Per-core layout: q/k/v are fed host-transposed as [DM, T] fp16 so the tensor
engine consumes them directly; accumulation is fp32 in PSUM.

Phase A streams k/v chunks: kh/vh projections, exp(kh), and the kv
reduction via an appended ones-column (column sums land in the same PSUM
bank). q and wq preload behind the k/v stream so phase B starts DMA-quiet.
The kv row-scale (1/S_k, on DVE) overlaps phase B's first qh matmuls.

Phase B runs one (ct, ch) tile per iteration: qh projection + q-softmax
over d as exp(qh - ln S) using a PE column-sum (sel_sum), ACT ln, PE
broadcast-subtract (sel_bc), and a second exp biased x32 against fp16
subnormals (kv carries the 1/32). The chain runs TWO iterations behind its
qh matmuls so the tensor engine never waits on ACT; the out matmul, DVE
fp32->fp16 copy, and the per-tile output DMA follow immediately, keeping
the tail to one tile's copy + 128KB store.
"""
import sys

import numpy as np

sys.path.insert(0, "/opt/trn_rl_repo")

import concourse.bacc as bacc
import concourse.mybir as mybir
from concourse import tile
from concourse.bass_utils import run_bass_kernel_spmd

F32 = mybir.dt.float32
BF16 = mybir.dt.bfloat16
FP16 = mybir.dt.float16
AFT = mybir.ActivationFunctionType
ALU = mybir.AluOpType

N, T, H, DM = 4, 4096, 16, 1024
C = 512          # columns (= 8 heads x 64) per core
NCORES = 8
TCH = T // 512   # 8 t-chunks of 512
DMC = DM // 128  # 8 contraction chunks
NCT = C // 128   # 4 col-tiles (head pairs)


def _patch_act_tables():
    """Steer both Exp and Ln onto the shared natural_log_exp_and_others ACT
    table (same 400-bucket precision) so the scheduler emits one table load
    instead of reloading on every Exp<->Ln switch (~1.3us each)."""
    if getattr(bacc, "_act_tables_patched", False):
        return
    orig = bacc.get_activation_tables

    def patched(arch):
        tables = dict(orig(arch))
        exp_t = mybir.ActivationFunctionType.Exp
        ln_t = mybir.ActivationFunctionType.Ln
        if "natural_log_exp_and_others" in tables:
            for name, funcs in tables.items():
                if name != "natural_log_exp_and_others":
                    tables[name] = funcs - {exp_t, ln_t}
        return tables

    bacc.get_activation_tables = patched
    bacc._act_tables_patched = True


def _build():
    _patch_act_tables()
    nc = bacc.Bacc("TRN2", target_bir_lowering=False, debug=False)
    # q/k/v come host-packed as [chunk, partition, dm, t] so every DMA piece
    # is 128 contiguous multi-KB descriptors (dm-strided sources crawl at
    # ~30GB/s; this layout sustains full HBM bandwidth)
    qT_d = nc.dram_tensor("qT", [TCH, 128, DMC, 512], FP16, kind="ExternalInput").ap()
    kT_d = nc.dram_tensor("kT", [TCH, 128, DMC, 512], FP16, kind="ExternalInput").ap()
    vT_d = nc.dram_tensor("vT", [TCH, 128, DMC, 512], FP16, kind="ExternalInput").ap()
    # weights host-packed as [partition, dm, col]
    wq_d = nc.dram_tensor("wq", [128, DMC, C], FP16, kind="ExternalInput").ap()
    wk_d = nc.dram_tensor("wk", [128, DMC, C], FP16, kind="ExternalInput").ap()
    wv_d = nc.dram_tensor("wv", [128, DMC, C], FP16, kind="ExternalInput").ap()
    sel_sum_d = nc.dram_tensor("sel_sum", [128, 2], FP16, kind="ExternalInput").ap()
    sel_bc_d = nc.dram_tensor("sel_bc", [2, 128], FP16, kind="ExternalInput").ap()
    neg4_d = nc.dram_tensor("neg4", [128, 1], F32, kind="ExternalInput").ap()
    outT_d = nc.dram_tensor("outT", [C, T], FP16, kind="ExternalOutput").ap()

    with tile.TileContext(nc) as tc:
        with (
            tc.tile_pool(name="weights", bufs=1) as wpool,
            tc.tile_pool(name="stream", bufs=2) as stream,
            tc.tile_pool(name="acts", bufs=3) as acts,
            tc.tile_pool(name="small", bufs=1) as small,
            tc.tile_pool(name="pswork", bufs=2, space="PSUM") as pswork,
            tc.tile_pool(name="psqh", bufs=4, space="PSUM") as psqh,
        ):
            wk_sb = wpool.tile([128, DMC, C], FP16, tag="wk")
            wv_sb = wpool.tile([128, DMC, C], FP16, tag="wv")
            wq_sb = wpool.tile([128, DMC, C], FP16, tag="wq")
            # all q chunks preload during phase A (64KB/partition)
            q_all = wpool.tile([128, TCH, DMC, 512], FP16, tag="qall")
            wk_r, wv_r, wq_r = wk_d, wv_d, wq_d
            sel_sum = small.tile([128, 2], FP16, tag="sel_sum")
            sel_bc = small.tile([2, 128], FP16, tag="sel_bc")
            neg4 = small.tile([128, 1], F32, tag="neg4")
            # tiny constants on the ACT queue before its exps start
            nc.scalar.dma_start(sel_sum[:], sel_sum_d[:])
            nc.scalar.dma_start(sel_bc[:], sel_bc_d[:])
            nc.scalar.dma_start(neg4[:], neg4_d[:])

            # kv block-diagonal stationary tiles for the phase-B out matmul
            kv_sb = [
                small.tile([128, 128], FP16, tag=f"kv{p}", name=f"kv{p}")
                for p in range(NCT)
            ]

            with tc.tile_pool(name="pskv", bufs=1, space="PSUM") as pskv:
                kvbank = [
                    pskv.tile([128, 260], F32, name=f"kvbank{b}") for b in range(2)
                ]
                kvps = [kvbank[p // 2][:, (p % 2) * 130 : (p % 2) * 130 + 130]
                        for p in range(NCT)]

                # ---- Phase A: k/v stream -> kv accumulation ----
                for ch in range(TCH):
                    ksb = stream.tile([128, DMC, 512], FP16, tag="k")
                    vsb = stream.tile([128, DMC, 512], FP16, tag="v")
                    if ch == 0:
                        # startup: smallest-first pieces in consumption order
                        # so the first matmul starts after ~0.25MB in flight,
                        # and the competing streams stay small early on.
                        # sync: wk + k. gpsimd: wv + v, then wq/q preloads.
                        nc.sync.dma_start(wk_sb[:, 0:1, :], wk_r[:, 0:1, :])
                        nc.sync.dma_start(ksb[:, 0:1, :], kT_d[0, :, 0:1, :])
                        nc.sync.dma_start(wk_sb[:, 1:2, :], wk_r[:, 1:2, :])
                        nc.sync.dma_start(ksb[:, 1:2, :], kT_d[0, :, 1:2, :])
                        nc.sync.dma_start(wk_sb[:, 2:8, :], wk_r[:, 2:8, :])
                        nc.sync.dma_start(ksb[:, 2:4, :], kT_d[0, :, 2:4, :])
                        nc.sync.dma_start(ksb[:, 4:8, :], kT_d[0, :, 4:8, :])
                        nc.gpsimd.dma_start(wv_sb[:, 0:1, :], wv_r[:, 0:1, :])
                        nc.gpsimd.dma_start(vsb[:, 0:1, :], vT_d[0, :, 0:1, :])
                        nc.gpsimd.dma_start(wv_sb[:, 1:2, :], wv_r[:, 1:2, :])
                        nc.gpsimd.dma_start(vsb[:, 1:2, :], vT_d[0, :, 1:2, :])
                        nc.gpsimd.dma_start(wv_sb[:, 2:8, :], wv_r[:, 2:8, :])
                        nc.gpsimd.dma_start(vsb[:, 2:4, :], vT_d[0, :, 2:4, :])
                        nc.gpsimd.dma_start(vsb[:, 4:8, :], vT_d[0, :, 4:8, :])
                        nc.gpsimd.dma_start(wq_sb[:, 0:4, :], wq_r[:, 0:4, :])
                        nc.gpsimd.dma_start(wq_sb[:, 4:8, :], wq_r[:, 4:8, :])
                    else:
                        nc.sync.dma_start(ksb[:, 0:4, :], kT_d[ch, :, 0:4, :])
                        nc.sync.dma_start(ksb[:, 4:8, :], kT_d[ch, :, 4:8, :])
                        nc.gpsimd.dma_start(vsb[:, 0:4, :], vT_d[ch, :, 0:4, :])
                        nc.gpsimd.dma_start(vsb[:, 4:8, :], vT_d[ch, :, 4:8, :])
                    # q preload trickles behind the v stream
                    nc.gpsimd.dma_start(q_all[:, ch, 0:4, :],
                                        qT_d[ch, :, 0:4, :])
                    nc.gpsimd.dma_start(q_all[:, ch, 4:8, :],
                                        qT_d[ch, :, 4:8, :])

                    for i in range(4):
                        ts128 = slice(i * 128, (i + 1) * 128)
                        kh_ps = pswork.tile([128, 512], F32, tag="work")
                        for dm in range(DMC):
                            nc.tensor.matmul(
                                kh_ps[:], ksb[:, dm, ts128], wk_sb[:, dm, :],
                                start=(dm == 0), stop=(dm == DMC - 1),
                            )
                        ek = acts.tile([128, 512], FP16, tag="ek")
                        nc.scalar.activation(ek[:], kh_ps[:], AFT.Exp)

                        vh_ps = pswork.tile([128, 512], F32, tag="work")
                        for dm in range(DMC):
                            nc.tensor.matmul(
                                vh_ps[:], vsb[:, dm, ts128], wv_sb[:, dm, :],
                                start=(dm == 0), stop=(dm == DMC - 1),
                            )
                        # vh_aug[p, pair, 0:128] = vh block; col 128 = 1.0
                        vh_aug = acts.tile([128, NCT, 130], FP16, tag="vh")
                        nc.vector.tensor_copy(
                            vh_aug[:, :, 0:128],
                            vh_ps[:].rearrange("p (c n) -> p c n", c=NCT),
                        )
                        # ones columns: 0*x + 1 (cheaper than a const DMA)
                        nc.vector.tensor_scalar(
                            vh_aug[:, :, 128:130],
                            vh_ps[:, 0:8].rearrange("p (c n) -> p c n", c=NCT),
                            0.0, 1.0, op0=ALU.mult, op1=ALU.add,
                        )

                        first = ch == 0 and i == 0
                        last = ch == TCH - 1 and i == 3
                        for p in range(NCT):
                            # start=True clears has_written for the whole PSUM
                            # bank; only the bank's first matmul may set it.
                            nc.tensor.matmul(
                                kvps[p][:],
                                ek[:, p * 128 : (p + 1) * 128],
                                vh_aug[:, p, :],
                                start=first and p % 2 == 0,
                                stop=last and p % 2 == 1,
                                skip_group_check=True,
                            )

                # kv rows scaled by 1/S_k (col 128 holds S_k); the DVE work
                # overlaps phase B's first qh matmuls
                for p in range(NCT):
                    rk = small.tile([128, 1], F32, tag=f"rk{p}", name=f"rk{p}")
                    with nc.allow_low_precision(reason="softmax reciprocal"):
                        nc.vector.reciprocal(rk[:], kvps[p][:, 128:129])
                    for half in range(2):
                        h64 = slice(half * 64, (half + 1) * 64)
                        o64 = slice((1 - half) * 64, (2 - half) * 64)
                        nc.vector.tensor_scalar(
                            kv_sb[p][h64, h64], kvps[p][h64, h64],
                            rk[h64, :], 0.03125,
                            op0=ALU.mult, op1=ALU.mult,
                        )
                        # off-diagonal cross-head block: zero via 0*x
                        nc.vector.tensor_scalar(
                            kv_sb[p][h64, o64], kvps[p][h64, o64],
                            0.0, None, op0=ALU.mult,
                        )

            # ---- Phase B: qh + q-softmax + out, chain 2 iters behind ----
            with tc.tile_pool(name="pssm", bufs=1, space="PSUM") as pssm:
                pending = []
                out_dma = [nc.sync, nc.gpsimd]
                stage = [None]

                def flush_out(j):
                    qh_ps, eq, ct, ch = pending.pop(0)
                    sq_ps = pssm.tile([2, 512], F32, tag="sm")
                    nc.tensor.matmul(sq_ps[:], sel_sum[:], eq[:], start=True,
                                     stop=True)
                    # lq = ln(S * e^-4) = ln(S) - 4: centered so fp16
                    # rounding of lq stays ~3e-4
                    lq = acts.tile([2, 512], FP16, tag="lq")
                    nc.scalar.activation(lq[:], sq_ps[:], AFT.Ln,
                                         scale=0.018315638888734179)
                    # qh_ps -= broadcast(ln S - 4): sel_bc holds -1 entries
                    nc.tensor.matmul(qh_ps[:], sel_bc[:], lq[:],
                                     start=False, stop=True,
                                     skip_group_check=True)
                    # bias = -4 + ln 32: undo centering, scale x32 to keep
                    # eq2 out of the fp16 subnormal range (kv carries /32)
                    eq2 = acts.tile([128, 512], FP16, tag="eq2")
                    nc.scalar.activation(eq2[:], qh_ps[:], AFT.Exp,
                                         bias=neg4[:])
                    o_ps = pswork.tile([128, 512], F32, tag="work")
                    nc.tensor.matmul(o_ps[:], kv_sb[ct][:], eq2[:],
                                     start=True, stop=True)
                    # stage two consecutive chunks (same ct, t-adjacent) so
                    # the output DMA moves 2KB-contiguous rows: ~4x the
                    # per-engine rate of single-tile 1KB stores
                    if ch % 2 == 0:
                        stage[0] = acts.tile([128, 2, 512], FP16, tag="osb",
                                             name="osb")
                    nc.vector.tensor_copy(stage[0][:, ch % 2, :], o_ps[:])
                    if ch % 2 == 1:
                        out_dma[j % 2].dma_start(
                            outT_d[ct * 128 : (ct + 1) * 128,
                                   (ch - 1) * 512 : (ch + 1) * 512],
                            stage[0][:],
                        )

                j = 0
                for ct in range(NCT):
                    for ch in range(TCH):
                        qh_ps = psqh.tile([128, 512], F32, tag="qh")
                        for dm in range(DMC):
                            nc.tensor.matmul(
                                qh_ps[:],
                                wq_sb[:, dm, ct * 128 : (ct + 1) * 128],
                                q_all[:, ch, dm, :],
                                start=(dm == 0), stop=(dm == DMC - 1),
                            )
                        eq = acts.tile([128, 512], FP16, tag="eq")
                        nc.scalar.activation(eq[:], qh_ps[:], AFT.Exp)
                        pending.append((qh_ps, eq, ct, ch))
                        if len(pending) >= 3:
                            flush_out(j)
                            j += 1
                while pending:
                    flush_out(j)
                    j += 1

    nc.compile()
    return nc


_NC_CACHE = None


def _get_nc():
    global _NC_CACHE
    if _NC_CACHE is None:
        _NC_CACHE = _build()
    return _NC_CACHE


def _make_in_maps(q, k, v, Wq, Wk, Wv):
    sel_sum = np.zeros((128, 2), np.float16)
    sel_sum[0:64, 0] = 1.0
    sel_sum[64:128, 1] = 1.0
    # negated: used to subtract the broadcast ln(S) inside the qh PSUM
    sel_bc = np.zeros((2, 128), np.float16)
    sel_bc[0, 0:64] = -1.0
    sel_bc[1, 64:128] = -1.0
    neg4 = np.full((128, 1), -4.0 + np.log(32.0), np.float32)

    f16 = np.float16

    def _pack_w(W, g):
        # [p, dm, col]: W[dm*128+p, g*C+c]
        Wg = np.asarray(W[:, g * C : (g + 1) * C], np.float32)
        return np.ascontiguousarray(
            Wg.reshape(DMC, 128, C).transpose(1, 0, 2)
        ).astype(f16)

    def _pack_x(x):
        # [ch, p, dm, t]: x[ch*512+t, dm*128+p]
        xa = np.asarray(x, np.float32)
        return np.ascontiguousarray(
            xa.reshape(TCH, 512, DMC, 128).transpose(0, 3, 2, 1)
        ).astype(f16)

    wq_r = [_pack_w(Wq, g) for g in range(2)]
    wk_r = [_pack_w(Wk, g) for g in range(2)]
    wv_r = [_pack_w(Wv, g) for g in range(2)]
    qT = [_pack_x(q[n]) for n in range(N)]
    kT = [_pack_x(k[n]) for n in range(N)]
    vT = [_pack_x(v[n]) for n in range(N)]

    in_maps = []
    for core in range(NCORES):
        n, g = core // 2, core % 2
        in_maps.append(
            {
                "qT": qT[n], "kT": kT[n], "vT": vT[n],
                "wq": wq_r[g], "wk": wk_r[g], "wv": wv_r[g],
                "sel_sum": sel_sum, "sel_bc": sel_bc, "neg4": neg4,
            }
        )
    return in_maps


def run(q, k, v, Wq, Wk, Wv, trace=False, trace_cores=None):
    nc = _get_nc()
    in_maps = _make_in_maps(q, k, v, Wq, Wk, Wv)
    res = run_bass_kernel_spmd(
        nc, in_maps, list(range(NCORES)), trace=trace, trace_cores=trace_cores
    )
    out = np.empty((N, T, H * 64), np.float32)
    for core in range(NCORES):
        n, g = core // 2, core % 2
        out[n, :, g * C : (g + 1) * C] = res.results[core]["outT"].T.astype(np.float32)
    return out, res


def kernel(q, k, v, Wq, Wk, Wv, mask_q=None, mask_attn=None, **_unused):
    out, _ = run(
        np.asarray(q, np.float32), np.asarray(k, np.float32),
        np.asarray(v, np.float32), np.asarray(Wq, np.float32),
        np.asarray(Wk, np.float32), np.asarray(Wv, np.float32),
    )
    return out
